# revision 32
# baseline (speedup 1.0000x reference)
"""Trainium2 Bass kernel for nn_Dictionary (vq_codebook): out = inp @ Q.T, Q from QR(weight+1e-8).

Strategy (per sharding_hint): data-parallel over batch B=131072 across 8 cores
(16384 rows each); Q.T replicated on every core (QR is tiny, computed on host).

Default mode "f16t2": the host transposes inp so the contraction dim i lands on
SBUF partitions with plain contiguous DMAs, and converts operands to fp16
(1 col/cycle on the PE at 2.4 GHz, fp32 PSUM accumulation -> rel L2 err
~3.6e-4; fp8 e4m3 fails the 2e-2 gate at 3.8e-2, so DoubleRow is unusable).
Per core: stationary = 128x128 blocks of Q.T (4 i-tiles x 4 j-tiles), moving =
[128i, 512b] slices of input chunks; each LDWEIGHTS serves 4 consecutive
matmuls (4 PSUM banks accumulate in parallel per j-tile while the previous 4
drain) - with stationary swapped every matmul the issue rate degrades 216 ->
259 ns. DVE/ACT cast-copy PSUM to fp16 out.T tiles ([128, 2048] -> 4KB DMA
packets); host transposes back. 34 warmup matmuls on a memset dummy, emitted
before the TileContext into the main block, run from ~7us so the HAM clock
gate sits at K=8/8 before real data lands (and cannot re-throttle in the gap
before the first chunk arrives). Q rides the sync HW queue, input the scalar
HW queue with 5-chunk prefetch depth (absorbs cross-core DMA-rate dips that
otherwise idle the PE >3.4us and re-throttle it mid-stream; never gpsimd:
software-DGE, ~60-100 GB/s), outputs sync except the final chunk's odd
j-tiles on scalar to halve the last flush.

Measured phase structure per NEFF execution (~130.5 us exec time):
~7us framework preamble + ~14us DMA-gated ramp (queues alive at ~9us, first
1.5MB at 90-183 GB/s early ring rate) overlapped by warmups, 110.7us matmul
stream at the exact 216 ns/MM fp16 floor (512 MMs, zero stalls), ~1-3us
flush, ~8.7us fixed walrus end-of-NEFF semaphore-reset trains.
"""

import os

import numpy as np

import concourse.bacc as bacc
import concourse.mybir as mybir
import concourse.tile as tile
from concourse.bass_utils import run_bass_kernel_spmd

N_CORES = 8
B = 131072
D = 512  # contraction dim i (NUM_BASIS)
J = 512  # output dim j (MOTION_DIM)
BC = B // N_CORES  # rows per core
P = 128
KT = D // P  # 4 i-tiles

MODE = os.environ.get("KERNEL_MODE", "f16t2")  # f16t3 | f16t2 | f16t | f16 | bf16 | f32r | f16x3

_DT_IN = {
    "f16": mybir.dt.float16,
    "f16t": mybir.dt.float16,
    "f16t2": mybir.dt.float16,
    "f16t3": mybir.dt.float16,
    "bf16": mybir.dt.bfloat16,
    "f32r": mybir.dt.float32r,
    "f16x3": mybir.dt.float16,
}

_compiled = {}
LAST_RESULTS = None  # BassKernelResults of the most recent run (for test.py)


def _np_in_dtype(mode):
    if mode in ("f16", "f16t", "f16t2", "f16t3", "f16x3"):
        return np.float16
    if mode == "bf16":
        import ml_dtypes

        return ml_dtypes.bfloat16
    return np.float32


def _build(mode, bc=BC, chunk=4096, ob=4):
    dt_in = _DT_IN[mode]
    hilo = mode.endswith("x3")
    nc = bacc.Bacc()
    if hilo:
        inpT_hi = nc.dram_tensor("inpT_hi", [D, bc], dt_in, kind="ExternalInput")
        inpT_lo = nc.dram_tensor("inpT_lo", [D, bc], dt_in, kind="ExternalInput")
        qT_hi = nc.dram_tensor("qT_hi", [D, J], dt_in, kind="ExternalInput")
        qT_lo = nc.dram_tensor("qT_lo", [D, J], dt_in, kind="ExternalInput")
        in_drams = [inpT_hi, inpT_lo]
        q_drams = [qT_hi, qT_lo]
    else:
        inpT = nc.dram_tensor("inpT", [D, bc], dt_in, kind="ExternalInput")
        qT = nc.dram_tensor("qT", [D, J], dt_in, kind="ExternalInput")
        in_drams = [inpT]
        q_drams = [qT]
    out = nc.dram_tensor("out", [bc, J], mybir.dt.float32, kind="ExternalOutput")

    BCk = bc
    CHUNK = chunk  # b-columns fetched per supertile DMA (1 MB in fp16)
    OB = ob  # b-tiles batched per output DMA instruction

    # Output viewed as [p, ob-groups, j] so one DMA stores OB b-tiles.
    out3 = out.rearrange("(g ob p) j -> g p ob j", p=P, ob=OB)

    with tile.TileContext(nc) as tc:
        with (
            tc.tile_pool(name="qpool", bufs=1) as qpool,
            tc.tile_pool(name="inpool", bufs=2) as inpool,
            tc.tile_pool(name="outpool", bufs=3) as outpool,
            tc.tile_pool(name="psum", bufs=7, space="PSUM") as psum_pool,
        ):
            # Q.T tiles [i=128, j=512], static for the whole kernel.
            qts = []
            for qi, qd in enumerate(q_drams):
                for it in range(KT):
                    qt_t = qpool.tile([P, J], dt_in, tag=f"qt{qi}_{it}")
                    nc.sync.dma_start(out=qt_t[:], in_=qd[it * P : (it + 1) * P, :])
                    qts.append(qt_t)

            ot = None
            for chunk in range(BCk // CHUNK):
                csl = slice(chunk * CHUNK, (chunk + 1) * CHUNK)
                sups = []  # supertiles per (input, i-tile)
                for ii, ind in enumerate(in_drams):
                    for it in range(KT):
                        sup = inpool.tile([P, CHUNK], dt_in, tag=f"sup{ii}_{it}")
                        # input loads ride the ACT HWDGE ring; output the SP ring
                        nc.scalar.dma_start(
                            out=sup[:], in_=ind[it * P : (it + 1) * P, csl]
                        )
                        sups.append(sup)
                for bt in range(CHUNK // P):
                    bsl = slice(bt * P, (bt + 1) * P)
                    ps = psum_pool.tile([P, J], mybir.dt.float32, tag="ps")
                    if hilo:
                        # out = hi@Qhi + hi@Qlo + lo@Qhi  (drop lo@Qlo)
                        passes = [(0, 0), (0, 1), (1, 0)]
                    else:
                        passes = [(0, 0)]
                    n_mm = len(passes) * KT
                    mm = 0
                    for ii, qi in passes:
                        for it in range(KT):
                            nc.tensor.matmul(
                                ps[:],
                                sups[ii * KT + it][:, bsl],
                                qts[qi * KT + it][:],
                                start=(mm == 0),
                                stop=(mm == n_mm - 1),
                            )
                            mm += 1
                    gbt = chunk * (CHUNK // P) + bt  # global b-tile index
                    if gbt % OB == 0:
                        ot = outpool.tile([P, OB, J], mybir.dt.float32, tag="ot")
                    # split PSUM->SBUF copies across DVE and ACT
                    if gbt % 2 == 0:
                        nc.vector.tensor_copy(out=ot[:, gbt % OB, :], in_=ps[:])
                    else:
                        nc.scalar.copy(out=ot[:, gbt % OB, :], in_=ps[:])
                    if gbt % OB == OB - 1:
                        nc.sync.dma_start(out=out3[gbt // OB], in_=ot[:])
    nc.compile()
    return nc


def _build_t(mode, bc=BC, chunk=2048, outw=4096, warmup_mms=0):
    """Transposed-output variant: PSUM holds [j, b] tiles (stationary = Q.T
    128x128 blocks, moving = inpT [i, b] slices), output written as
    outT [J, bc] fp16 with wide per-partition runs, host transposes back.
    Halves output HBM traffic and keeps DMA packets large (>= 4 KB)."""
    dt_in = _DT_IN[mode]
    assert dt_in == mybir.dt.float16
    nc = bacc.Bacc()
    inpT = nc.dram_tensor("inpT", [D, bc], dt_in, kind="ExternalInput")
    qT = nc.dram_tensor("qT", [D, J], dt_in, kind="ExternalInput")
    outT = nc.dram_tensor("outT", [J, bc], mybir.dt.float16, kind="ExternalOutput")

    NB = 512  # moving free dim per matmul (one PSUM bank of fp32)
    JT = J // P  # 4 j-tiles

    # Input chunk schedule: uniform chunks (leading small chunk measured worse).
    plan = []
    rem = bc
    while rem > 0:
        c = min(chunk, rem)
        plan.append(c)
        rem -= c

    # Output group schedule: small groups at both ends (early first store,
    # short final flush), wide in the middle for large DMA packets.
    ow_plan = []
    rem = bc
    if bc >= 4 * outw:
        for c in (1024, 1024, 2048):
            ow_plan.append(c)
            rem -= c
    tail = [1024, 1024, 2048] if bc >= 4 * outw else []
    rem -= sum(tail)
    while rem > 0:
        c = min(outw, rem)
        ow_plan.append(c)
        rem -= c
    ow_plan.extend(reversed(tail))
    assert sum(ow_plan) == bc and all(w % 512 == 0 for w in ow_plan)
    # column index -> (group_idx, offset, width)
    col2grp = {}
    base = 0
    for gi, w in enumerate(ow_plan):
        for off in range(0, w, 512):
            col2grp[base + off] = (gi, off, w)
        base += w
    grp_base = {}
    base = 0
    for gi, w in enumerate(ow_plan):
        grp_base[gi] = base
        base += w

    with tile.TileContext(nc) as tc:
        with (
            tc.tile_pool(name="qpool", bufs=1) as qpool,
            tc.tile_pool(name="inpool", bufs=3) as inpool,
            tc.tile_pool(name="outpool", bufs=2) as outpool,
            tc.tile_pool(name="psum", bufs=8, space="PSUM") as psum_pool,
            tc.tile_pool(name="warm", bufs=1) as warm_pool,
            tc.tile_pool(name="warmps", bufs=1, space="PSUM") as warmps_pool,
        ):
            # Q.T rows for i-tile `it`: [128i, 512j]; stationary blocks are
            # 128-column slices qts[it][:, jt*128:(jt+1)*128]. (Dedicated
            # contiguous [128,128] weight tiles measured WORSE: 137.2us.)
            qts = []
            for it in range(KT):
                qt_t = qpool.tile([P, J], dt_in, tag=f"qt{it}")
                nc.gpsimd.dma_start(out=qt_t[:], in_=qT[it * P : (it + 1) * P, :])
                qts.append(qt_t)

            if warmup_mms:
                # Warmup matmuls on the (tiny, early-arriving) qT tiles: keeps
                # the PE HAM busy while the first input chunk streams in, so
                # real matmuls start un-throttled. Result bank is never read.
                wps = warmps_pool.tile([P, NB], mybir.dt.float32, tag="wps")
                for wi in range(warmup_mms):
                    nc.tensor.matmul(
                        wps[:],
                        qts[0][:, :P],
                        qts[0][:],
                        start=(wi == 0),
                        stop=(wi == warmup_mms - 1),
                    )

            ots = [None] * JT
            col_base = 0
            for chunk_i, csz in enumerate(plan):
                csl = slice(col_base, col_base + csz)
                sups = []
                for it in range(KT):
                    sup = inpool.tile([P, csz], dt_in, tag=f"sup{it}")
                    nc.scalar.dma_start(
                        out=sup[:], in_=inpT[it * P : (it + 1) * P, csl]
                    )
                    sups.append(sup)
                for bn in range(csz // NB):
                    col0 = col_base + bn * NB
                    gi, goff, gw = col2grp[col0]
                    if goff == 0:
                        for jt in range(JT):
                            ots[jt] = outpool.tile(
                                [P, outw],
                                mybir.dt.float16,
                                tag=f"ot{jt}",
                                name=f"ot{jt}",
                            )
                    osl = slice(goff, goff + NB)
                    bsl = slice(bn * NB, (bn + 1) * NB)
                    for jt in range(JT):
                        ps = psum_pool.tile([P, NB], mybir.dt.float32, tag="ps")
                        for it in range(KT):
                            nc.tensor.matmul(
                                ps[:],
                                qts[it][:, jt * P : (jt + 1) * P],
                                sups[it][:, bsl],
                                start=(it == 0),
                                stop=(it == KT - 1),
                            )
                        if jt % 2 == 0:
                            nc.vector.tensor_copy(out=ots[jt][:, osl], in_=ps[:])
                        else:
                            nc.scalar.copy(out=ots[jt][:, osl], in_=ps[:])
                    if goff + NB == gw:
                        g0 = grp_base[gi]
                        for jt in range(JT):
                            nc.sync.dma_start(
                                out=outT[jt * P : (jt + 1) * P, g0 : g0 + gw],
                                in_=ots[jt][:, :gw],
                            )
                col_base += csz
    nc.compile()
    return nc


def _build_t2(
    mode,
    bc=BC,
    chunk=None,
    warmups=None,
):
    """v2 of the transposed-output kernel. Differences from _build_t:

    - Weight reuse: per (chunk, jt, it) ONE stationary Q block feeds G
      consecutive matmuls (one per 512-col b-block, each accumulating into
      its own PSUM bank), instead of swapping stationary every matmul.
    - Warmup matmuls on a memset dummy tile run from t~0 (no DMA
      dependency), so the HAM clock-gate releases before real matmuls and
      the PE isn't idle during the input DMA cold-start.
    - Output: one [128, chunk] fp16 tile per (chunk, jt), stored with a
      single DMA (4 KB/partition contiguous); first/last chunks smaller to
      shorten ramp and final flush.
    """
    dt_in = _DT_IN[mode]
    assert dt_in == mybir.dt.float16
    if chunk is None:
        chunk = int(os.environ.get("KCHUNK", "2048"))
    if warmups is None:
        warmups = int(os.environ.get("KWARM", "34"))
    chl = int(os.environ.get("KCHL", "1024"))
    WNB = int(os.environ.get("KWNB", "256"))  # warmup moving width
    ibufs = int(os.environ.get("KIBUFS", "5"))  # input prefetch depth (chunks)

    nc = bacc.Bacc()
    inpT = nc.dram_tensor("inpT", [D, bc], dt_in, kind="ExternalInput")
    qT = nc.dram_tensor("qT", [D, J], dt_in, kind="ExternalInput")
    outT = nc.dram_tensor("outT", [J, bc], mybir.dt.float16, kind="ExternalOutput")

    NB = 512  # moving free dim per matmul (one PSUM bank)
    JT = J // P  # 4 j-tiles

    # chunk plan: measured-optimal head is a single 1024 chunk (the stream
    # start is gated by ~1.5MB of DMA however it is sliced; finer slicing
    # only buys LDWEIGHTS exposure), wide middle, 1024 last (fast flush)
    head = [int(x) for x in os.environ.get("KHEAD", "1024").split(",") if x]
    plan = []
    rem = bc
    for c in head:
        if rem - c > chl:
            plan.append(c)
            rem -= c
    while rem > (chl if chl else 0):
        c = min(chunk, rem - chl)
        plan.append(c)
        rem -= c
    if rem:
        plan.append(rem)
    assert sum(plan) == bc and all(c % NB == 0 for c in plan), plan

    if warmups:
        # Pre-TileContext warmup matmuls, emitted straight into the `main`
        # block so the PE starts them right after the framework preamble
        # (~5.8us), well before the Tile body's first DMA lands. They warm
        # the HAM clock gate to K=8/8 (2.4 GHz) so real matmuls start
        # un-throttled. No cross-engine sync on purpose: the operand values
        # are irrelevant (the PSUM bank is never read) so reading SBUF
        # before/while gpsimd memsets it is harmless.
        wsb = nc.alloc_sbuf_tensor("wsb", [P, WNB], dt_in)
        nc.gpsimd.memset(wsb[:, :], 0)
        wps = nc.alloc_psum_tensor("wps", [P, WNB], mybir.dt.float32)
        for wi in range(warmups):
            nc.tensor.matmul(
                wps[:, :],
                wsb[:, :P],
                wsb[:, :],
                start=(wi == 0),
                stop=(wi == warmups - 1),
            )

    with tile.TileContext(nc) as tc:
        with (
            tc.tile_pool(name="qpool", bufs=1) as qpool,
            tc.tile_pool(name="inpool", bufs=ibufs) as inpool,
            tc.tile_pool(name="outpool", bufs=2) as outpool,
            tc.tile_pool(name="psum", bufs=7, space="PSUM") as psum_pool,
        ):
            # Q.T rows for i-tile `it`: [128i, 512j]; stationary blocks are
            # 128-col slices. Ride the sync HW queue (idle early) in parallel
            # with the input ring. NOT gpsimd (software-DGE, slow).
            qts = []
            for it in range(KT):
                qt_t = qpool.tile([P, J], dt_in, tag=f"qt{it}", name=f"qt{it}")
                nc.sync.dma_start(out=qt_t[:], in_=qT[it * P : (it + 1) * P, :])
                qts.append(qt_t)

            # All input on the scalar ring: per-ring FIFO keeps arrival
            # order == consumption order, and the graduated head chunks keep
            # the first slices from queueing behind bulk prefetch.
            col_base = 0
            for ci, csz in enumerate(plan):
                G = csz // NB
                csl = slice(col_base, col_base + csz)
                sups = []
                for it in range(KT):
                    sup = inpool.tile([P, csz], dt_in, tag=f"sup{it}", name=f"sup{it}")
                    nc.scalar.dma_start(
                        out=sup[:], in_=inpT[it * P : (it + 1) * P, csl]
                    )
                    sups.append(sup)
                for jt in range(JT):
                    pss = [
                        psum_pool.tile([P, NB], mybir.dt.float32, tag="ps", name="ps")
                        for _ in range(G)
                    ]
                    ot = outpool.tile(
                        [P, csz], mybir.dt.float16, tag=f"ot{jt}", name=f"ot{jt}"
                    )
                    for it in range(KT):
                        # one stationary load serves G consecutive matmuls
                        for g in range(G):
                            nc.tensor.matmul(
                                pss[g][:],
                                qts[it][:, jt * P : (jt + 1) * P],
                                sups[it][:, g * NB : (g + 1) * NB],
                                start=(it == 0),
                                stop=(it == KT - 1),
                            )
                    for g in range(G):
                        # split PSUM->SBUF cast-copies across DVE and ACT
                        if g % 2 == 0:
                            nc.vector.tensor_copy(
                                out=ot[:, g * NB : (g + 1) * NB], in_=pss[g][:]
                            )
                        else:
                            nc.scalar.copy(
                                out=ot[:, g * NB : (g + 1) * NB], in_=pss[g][:]
                            )
                    # outputs ride the sync HW ring; the LAST chunk's odd-jt
                    # flushes ride scalar (input is long done), halving the
                    # end-of-stream flush latency
                    last = ci == len(plan) - 1
                    oeng = nc.scalar if (last and jt % 2 == 1) else nc.sync
                    oeng.dma_start(
                        out=outT[jt * P : (jt + 1) * P, csl], in_=ot[:]
                    )
                col_base += csz
    nc.compile()
    return nc


def _build_t3(mode, bc=BC):
    """v3 of the transposed-output kernel: host-interleaved input blocks.

    Host stores the input as blocks of 512 b-columns with all 4 i-slices
    contiguous per partition: inp_blk[p, b, it, c] = inpT[it*128+p, b*512+c],
    so EVERY input DMA moves 4 KB/partition contiguous runs (the early DMA
    rate is packet-size-bound: 1 KB packets reach only ~130 GB/s, 4 KB
    ~210 GB/s). One transfer per block; scalar-ring FIFO; tile-pool bufs
    bound the prefetch depth. Q is likewise a single interleaved transfer.

    Matmul schedule: pairs of blocks (G=2 is enough to hide LDWEIGHTS) with
    NB=512; the first/last blocks run alone with NB=256 (still G=2) so the
    stream can start on half a megabyte and the final flush is small.
    """
    dt_in = _DT_IN[mode]
    assert dt_in == mybir.dt.float16
    warmups = int(os.environ.get("KWARM", "16"))
    WNB = int(os.environ.get("KWNB", "256"))
    n_sing_head = int(os.environ.get("KSINGH", "2"))
    n_sing_tail = int(os.environ.get("KSINGT", "2"))
    BUFS = int(os.environ.get("KBUFS", "8"))

    NBLK = bc // 512  # 32 blocks of 512 b-columns
    assert (NBLK - n_sing_head - n_sing_tail) % 2 == 0

    nc = bacc.Bacc()
    # interleaved layouts (built on host)
    inp_blk = nc.dram_tensor("inp_blk", [P, NBLK * KT * 512], dt_in, kind="ExternalInput")
    q_blk = nc.dram_tensor("q_blk", [P, KT * J], dt_in, kind="ExternalInput")
    outT = nc.dram_tensor("outT", [J, bc], mybir.dt.float16, kind="ExternalOutput")

    JT = J // P

    if warmups:
        wsb = nc.alloc_sbuf_tensor("wsb", [P, WNB], dt_in)
        nc.gpsimd.memset(wsb[:, :], 0)
        wps = nc.alloc_psum_tensor("wps", [P, WNB], mybir.dt.float32)
        for wi in range(warmups):
            nc.tensor.matmul(
                wps[:, :],
                wsb[:, :P],
                wsb[:, :],
                start=(wi == 0),
                stop=(wi == warmups - 1),
            )

    # chunk plan over blocks: singles at head/tail, pairs in the middle
    chunks = [[b] for b in range(n_sing_head)]
    b = n_sing_head
    while b < NBLK - n_sing_tail:
        chunks.append([b, b + 1])
        b += 2
    chunks.extend([[b] for b in range(NBLK - n_sing_tail, NBLK)])

    with tile.TileContext(nc) as tc:
        with (
            tc.tile_pool(name="qpool", bufs=1) as qpool,
            tc.tile_pool(name="inpool", bufs=BUFS) as inpool,
            tc.tile_pool(name="outpool", bufs=2) as outpool,
            tc.tile_pool(name="psum", bufs=7, space="PSUM") as psum_pool,
        ):
            # Q: ONE interleaved transfer on the sync HW queue: 4KB/partition
            # contiguous run -> 4KB packets (per-i-tile slices would be 1KB
            # packets at ~50 GB/s). Never gpsimd: software-DGE, slow.
            qt = qpool.tile([P, KT * J], dt_in, tag="qt", name="qt")
            nc.sync.dma_start(out=qt[:], in_=q_blk[:, :])

            # all input transfers up-front in block order; pool bufs bound
            # the in-flight prefetch window
            blks = []
            for bi in range(NBLK):
                t = inpool.tile([P, KT * 512], dt_in, tag="blk", name="blk")
                nc.scalar.dma_start(
                    out=t[:], in_=inp_blk[:, bi * KT * 512 : (bi + 1) * KT * 512]
                )
                blks.append(t)

            # output groups: wide (2048 cols -> 4KB packets) in the middle,
            # tapered at the end so the final flush after the last matmul is
            # small. Group boundaries align with chunk boundaries.
            ow_plan = [4, 4, 4, 4, 4, 4, 4, 2, 1, 1]  # in blocks; sums to 32
            assert sum(ow_plan) == NBLK
            blk2grp = {}
            gb = 0
            for gi, w in enumerate(ow_plan):
                for off in range(w):
                    blk2grp[gb + off] = (gi, off, w)
                gb += w
            grp_base = {}
            gb = 0
            for gi, w in enumerate(ow_plan):
                grp_base[gi] = gb
                gb += w
            ots = [None] * JT

            copy_i = 0
            for chunk in chunks:
                csz = 512 * len(chunk)
                nb = csz // 2 if len(chunk) == 1 else 512  # NB=256 singles, 512 pairs
                for jt in range(JT):
                    pss = [
                        psum_pool.tile([P, nb], mybir.dt.float32, tag="ps", name="ps")
                        for _ in range(2)
                    ]
                    gi, goff, gw = blk2grp[chunk[0]]
                    if goff == 0 and jt == 0:
                        for j2 in range(JT):
                            ots[j2] = outpool.tile(
                                [P, gw * 512],
                                mybir.dt.float16,
                                tag=f"ot{j2}",
                                name=f"ot{j2}",
                            )
                    for it in range(KT):
                        for g in range(2):
                            if len(chunk) == 1:
                                mv = blks[chunk[0]][:, it * 512 + g * nb : it * 512 + (g + 1) * nb]
                            else:
                                mv = blks[chunk[g]][:, it * 512 : (it + 1) * 512]
                            nc.tensor.matmul(
                                pss[g][:],
                                qt[:, it * J + jt * P : it * J + (jt + 1) * P],
                                mv,
                                start=(it == 0),
                                stop=(it == KT - 1),
                            )
                    for g in range(2):
                        osl = slice(goff * 512 + g * nb, goff * 512 + (g + 1) * nb)
                        # alternate engines by a global counter so DVE and
                        # ACT each carry half the PSUM-drain load
                        if copy_i % 2 == 0:
                            nc.vector.tensor_copy(out=ots[jt][:, osl], in_=pss[g][:])
                        else:
                            nc.scalar.copy(out=ots[jt][:, osl], in_=pss[g][:])
                        copy_i += 1
                    if goff + len(chunk) == gw:
                        g0 = grp_base[gi] * 512
                        nc.sync.dma_start(
                            out=outT[jt * P : (jt + 1) * P, g0 : g0 + gw * 512],
                            in_=ots[jt][:, : gw * 512],
                        )
    nc.compile()
    return nc


def _get_nc(mode):
    if mode not in _compiled:
        if mode == "f16t3":
            _compiled[mode] = _build_t3(mode)
        elif mode == "f16t2":
            _compiled[mode] = _build_t2(mode)
        elif mode == "f16t":
            _compiled[mode] = _build_t(mode)
        else:
            _compiled[mode] = _build(mode)
    return _compiled[mode]


def kernel(inp: np.ndarray, weight: np.ndarray) -> np.ndarray:
    global LAST_RESULTS
    mode = MODE
    nc = _get_nc(mode)

    w = np.asarray(weight, dtype=np.float32) + np.float32(1e-8)
    Q = np.linalg.qr(w)[0].astype(np.float32)  # [J, D] == [512, 512]
    np_dt = _np_in_dtype(mode)

    inp = np.asarray(inp, dtype=np.float32)
    inpT = inp.T  # [D, B] view

    QT = Q.T  # QT[i, j] = Q[j, i]
    in_maps = []
    if mode.endswith("x3"):
        qt_hi = QT.astype(np_dt)
        qt_lo = (QT - qt_hi.astype(np.float32)).astype(np_dt)
        for c in range(N_CORES):
            sl = inpT[:, c * BC : (c + 1) * BC].astype(np.float32)
            hi = sl.astype(np_dt)
            lo = (sl - hi.astype(np.float32)).astype(np_dt)
            in_maps.append(
                {"inpT_hi": hi, "inpT_lo": lo, "qT_hi": qt_hi, "qT_lo": qt_lo}
            )
    elif mode == "f16t3":
        # interleaved layouts: q_blk[p, it*512+j] = QT[it*128+p, j];
        # inp_blk[p, (b*4+it)*512+c] = inpT[it*128+p, b*512+c]
        q_blk = np.ascontiguousarray(
            QT.astype(np_dt).reshape(KT, P, J).transpose(1, 0, 2)
        ).reshape(P, KT * J)
        for c in range(N_CORES):
            sl = inpT[:, c * BC : (c + 1) * BC].astype(np_dt)  # [D, BC]
            blk = np.ascontiguousarray(
                sl.reshape(KT, P, BC // 512, 512).transpose(1, 2, 0, 3)
            ).reshape(P, BC * KT)
            in_maps.append({"inp_blk": blk, "q_blk": q_blk})
    else:
        qt16 = np.ascontiguousarray(QT).astype(np_dt)
        for c in range(N_CORES):
            in_maps.append(
                {"inpT": inpT[:, c * BC : (c + 1) * BC].astype(np_dt), "qT": qt16}
            )

    # First execution of a freshly compiled NEFF occasionally dies with
    # NRT_EXEC_UNIT_UNRECOVERABLE (transient, esp. with profiling on);
    # a straight retry has always succeeded.
    last_exc = None
    for _attempt in range(3):
        try:
            res = run_bass_kernel_spmd(nc, in_maps, list(range(N_CORES)))
            break
        except Exception as e:  # noqa: BLE001
            last_exc = e
            import time as _time

            _time.sleep(2.0)
    else:
        raise last_exc
    LAST_RESULTS = res
    if mode in ("f16t", "f16t2", "f16t3"):
        out = np.empty((B, J), dtype=np.float32)
        for c in range(N_CORES):
            # outT [J, BC] fp16 -> out rows [c*BC:(c+1)*BC] fp32
            out[c * BC : (c + 1) * BC, :] = res.results[c]["outT"].T
        return out
    return np.concatenate([res.results[c]["out"] for c in range(N_CORES)], axis=0)



# revision 33
# speedup vs baseline: 1.0013x; 1.0013x over previous
"""Trainium2 Bass kernel for nn_Dictionary (vq_codebook): out = inp @ Q.T, Q from QR(weight+1e-8).

Strategy (per sharding_hint): data-parallel over batch B=131072 across 8 cores
(16384 rows each); Q.T replicated on every core (QR is tiny, computed on host).

Default mode "f16t2": the host transposes inp so the contraction dim i lands on
SBUF partitions with plain contiguous DMAs, and converts operands to fp16
(1 col/cycle on the PE at 2.4 GHz, fp32 PSUM accumulation -> rel L2 err
~3.6e-4; fp8 e4m3 fails the 2e-2 gate at 3.8e-2, so DoubleRow is unusable).
Per core: stationary = 128x128 blocks of Q.T (4 i-tiles x 4 j-tiles), moving =
[128i, 512b] slices of input chunks; each LDWEIGHTS serves 4 consecutive
matmuls (4 PSUM banks accumulate in parallel per j-tile while the previous 4
drain) - with stationary swapped every matmul the issue rate degrades 216 ->
259 ns. DVE/ACT cast-copy PSUM to fp16 out.T tiles ([128, 2048] -> 4KB DMA
packets); host transposes back. 34 warmup matmuls on a memset dummy, emitted
before the TileContext into the main block, run from ~7us so the HAM clock
gate sits at K=8/8 before real data lands (and cannot re-throttle in the gap
before the first chunk arrives). Q rides the sync HW queue, input the scalar
HW queue with 5-chunk prefetch depth (absorbs cross-core DMA-rate dips that
otherwise idle the PE >3.4us and re-throttle it mid-stream; never gpsimd:
software-DGE, ~60-100 GB/s), outputs sync except the final chunk's odd
j-tiles on scalar to halve the last flush.

Measured phase structure per NEFF execution (~130.5 us exec time):
~7us framework preamble + ~14us DMA-gated ramp (queues alive at ~9us, first
1.5MB at 90-183 GB/s early ring rate) overlapped by warmups, 110.7us matmul
stream at the exact 216 ns/MM fp16 floor (512 MMs, zero stalls), ~1-3us
flush, ~8.7us fixed walrus end-of-NEFF semaphore-reset trains.
"""

import os

import numpy as np

import concourse.bacc as bacc
import concourse.mybir as mybir
import concourse.tile as tile
from concourse.bass_utils import run_bass_kernel_spmd

N_CORES = 8
B = 131072
D = 512  # contraction dim i (NUM_BASIS)
J = 512  # output dim j (MOTION_DIM)
BC = B // N_CORES  # rows per core
P = 128
KT = D // P  # 4 i-tiles

MODE = os.environ.get("KERNEL_MODE", "f16t2")  # f16t3 | f16t2 | f16t | f16 | bf16 | f32r | f16x3

_DT_IN = {
    "f16": mybir.dt.float16,
    "f16t": mybir.dt.float16,
    "f16t2": mybir.dt.float16,
    "f16t3": mybir.dt.float16,
    "bf16": mybir.dt.bfloat16,
    "f32r": mybir.dt.float32r,
    "f16x3": mybir.dt.float16,
}

_compiled = {}
LAST_RESULTS = None  # BassKernelResults of the most recent run (for test.py)


def _np_in_dtype(mode):
    if mode in ("f16", "f16t", "f16t2", "f16t3", "f16x3"):
        return np.float16
    if mode == "bf16":
        import ml_dtypes

        return ml_dtypes.bfloat16
    return np.float32


def _build(mode, bc=BC, chunk=4096, ob=4):
    dt_in = _DT_IN[mode]
    hilo = mode.endswith("x3")
    nc = bacc.Bacc()
    if hilo:
        inpT_hi = nc.dram_tensor("inpT_hi", [D, bc], dt_in, kind="ExternalInput")
        inpT_lo = nc.dram_tensor("inpT_lo", [D, bc], dt_in, kind="ExternalInput")
        qT_hi = nc.dram_tensor("qT_hi", [D, J], dt_in, kind="ExternalInput")
        qT_lo = nc.dram_tensor("qT_lo", [D, J], dt_in, kind="ExternalInput")
        in_drams = [inpT_hi, inpT_lo]
        q_drams = [qT_hi, qT_lo]
    else:
        inpT = nc.dram_tensor("inpT", [D, bc], dt_in, kind="ExternalInput")
        qT = nc.dram_tensor("qT", [D, J], dt_in, kind="ExternalInput")
        in_drams = [inpT]
        q_drams = [qT]
    out = nc.dram_tensor("out", [bc, J], mybir.dt.float32, kind="ExternalOutput")

    BCk = bc
    CHUNK = chunk  # b-columns fetched per supertile DMA (1 MB in fp16)
    OB = ob  # b-tiles batched per output DMA instruction

    # Output viewed as [p, ob-groups, j] so one DMA stores OB b-tiles.
    out3 = out.rearrange("(g ob p) j -> g p ob j", p=P, ob=OB)

    with tile.TileContext(nc) as tc:
        with (
            tc.tile_pool(name="qpool", bufs=1) as qpool,
            tc.tile_pool(name="inpool", bufs=2) as inpool,
            tc.tile_pool(name="outpool", bufs=3) as outpool,
            tc.tile_pool(name="psum", bufs=7, space="PSUM") as psum_pool,
        ):
            # Q.T tiles [i=128, j=512], static for the whole kernel.
            qts = []
            for qi, qd in enumerate(q_drams):
                for it in range(KT):
                    qt_t = qpool.tile([P, J], dt_in, tag=f"qt{qi}_{it}")
                    nc.sync.dma_start(out=qt_t[:], in_=qd[it * P : (it + 1) * P, :])
                    qts.append(qt_t)

            ot = None
            for chunk in range(BCk // CHUNK):
                csl = slice(chunk * CHUNK, (chunk + 1) * CHUNK)
                sups = []  # supertiles per (input, i-tile)
                for ii, ind in enumerate(in_drams):
                    for it in range(KT):
                        sup = inpool.tile([P, CHUNK], dt_in, tag=f"sup{ii}_{it}")
                        # input loads ride the ACT HWDGE ring; output the SP ring
                        nc.scalar.dma_start(
                            out=sup[:], in_=ind[it * P : (it + 1) * P, csl]
                        )
                        sups.append(sup)
                for bt in range(CHUNK // P):
                    bsl = slice(bt * P, (bt + 1) * P)
                    ps = psum_pool.tile([P, J], mybir.dt.float32, tag="ps")
                    if hilo:
                        # out = hi@Qhi + hi@Qlo + lo@Qhi  (drop lo@Qlo)
                        passes = [(0, 0), (0, 1), (1, 0)]
                    else:
                        passes = [(0, 0)]
                    n_mm = len(passes) * KT
                    mm = 0
                    for ii, qi in passes:
                        for it in range(KT):
                            nc.tensor.matmul(
                                ps[:],
                                sups[ii * KT + it][:, bsl],
                                qts[qi * KT + it][:],
                                start=(mm == 0),
                                stop=(mm == n_mm - 1),
                            )
                            mm += 1
                    gbt = chunk * (CHUNK // P) + bt  # global b-tile index
                    if gbt % OB == 0:
                        ot = outpool.tile([P, OB, J], mybir.dt.float32, tag="ot")
                    # split PSUM->SBUF copies across DVE and ACT
                    if gbt % 2 == 0:
                        nc.vector.tensor_copy(out=ot[:, gbt % OB, :], in_=ps[:])
                    else:
                        nc.scalar.copy(out=ot[:, gbt % OB, :], in_=ps[:])
                    if gbt % OB == OB - 1:
                        nc.sync.dma_start(out=out3[gbt // OB], in_=ot[:])
    nc.compile()
    return nc


def _build_t(mode, bc=BC, chunk=2048, outw=4096, warmup_mms=0):
    """Transposed-output variant: PSUM holds [j, b] tiles (stationary = Q.T
    128x128 blocks, moving = inpT [i, b] slices), output written as
    outT [J, bc] fp16 with wide per-partition runs, host transposes back.
    Halves output HBM traffic and keeps DMA packets large (>= 4 KB)."""
    dt_in = _DT_IN[mode]
    assert dt_in == mybir.dt.float16
    nc = bacc.Bacc()
    inpT = nc.dram_tensor("inpT", [D, bc], dt_in, kind="ExternalInput")
    qT = nc.dram_tensor("qT", [D, J], dt_in, kind="ExternalInput")
    outT = nc.dram_tensor("outT", [J, bc], mybir.dt.float16, kind="ExternalOutput")

    NB = 512  # moving free dim per matmul (one PSUM bank of fp32)
    JT = J // P  # 4 j-tiles

    # Input chunk schedule: uniform chunks (leading small chunk measured worse).
    plan = []
    rem = bc
    while rem > 0:
        c = min(chunk, rem)
        plan.append(c)
        rem -= c

    # Output group schedule: small groups at both ends (early first store,
    # short final flush), wide in the middle for large DMA packets.
    ow_plan = []
    rem = bc
    if bc >= 4 * outw:
        for c in (1024, 1024, 2048):
            ow_plan.append(c)
            rem -= c
    tail = [1024, 1024, 2048] if bc >= 4 * outw else []
    rem -= sum(tail)
    while rem > 0:
        c = min(outw, rem)
        ow_plan.append(c)
        rem -= c
    ow_plan.extend(reversed(tail))
    assert sum(ow_plan) == bc and all(w % 512 == 0 for w in ow_plan)
    # column index -> (group_idx, offset, width)
    col2grp = {}
    base = 0
    for gi, w in enumerate(ow_plan):
        for off in range(0, w, 512):
            col2grp[base + off] = (gi, off, w)
        base += w
    grp_base = {}
    base = 0
    for gi, w in enumerate(ow_plan):
        grp_base[gi] = base
        base += w

    with tile.TileContext(nc) as tc:
        with (
            tc.tile_pool(name="qpool", bufs=1) as qpool,
            tc.tile_pool(name="inpool", bufs=3) as inpool,
            tc.tile_pool(name="outpool", bufs=2) as outpool,
            tc.tile_pool(name="psum", bufs=8, space="PSUM") as psum_pool,
            tc.tile_pool(name="warm", bufs=1) as warm_pool,
            tc.tile_pool(name="warmps", bufs=1, space="PSUM") as warmps_pool,
        ):
            # Q.T rows for i-tile `it`: [128i, 512j]; stationary blocks are
            # 128-column slices qts[it][:, jt*128:(jt+1)*128]. (Dedicated
            # contiguous [128,128] weight tiles measured WORSE: 137.2us.)
            qts = []
            for it in range(KT):
                qt_t = qpool.tile([P, J], dt_in, tag=f"qt{it}")
                nc.gpsimd.dma_start(out=qt_t[:], in_=qT[it * P : (it + 1) * P, :])
                qts.append(qt_t)

            if warmup_mms:
                # Warmup matmuls on the (tiny, early-arriving) qT tiles: keeps
                # the PE HAM busy while the first input chunk streams in, so
                # real matmuls start un-throttled. Result bank is never read.
                wps = warmps_pool.tile([P, NB], mybir.dt.float32, tag="wps")
                for wi in range(warmup_mms):
                    nc.tensor.matmul(
                        wps[:],
                        qts[0][:, :P],
                        qts[0][:],
                        start=(wi == 0),
                        stop=(wi == warmup_mms - 1),
                    )

            ots = [None] * JT
            col_base = 0
            for chunk_i, csz in enumerate(plan):
                csl = slice(col_base, col_base + csz)
                sups = []
                for it in range(KT):
                    sup = inpool.tile([P, csz], dt_in, tag=f"sup{it}")
                    nc.scalar.dma_start(
                        out=sup[:], in_=inpT[it * P : (it + 1) * P, csl]
                    )
                    sups.append(sup)
                for bn in range(csz // NB):
                    col0 = col_base + bn * NB
                    gi, goff, gw = col2grp[col0]
                    if goff == 0:
                        for jt in range(JT):
                            ots[jt] = outpool.tile(
                                [P, outw],
                                mybir.dt.float16,
                                tag=f"ot{jt}",
                                name=f"ot{jt}",
                            )
                    osl = slice(goff, goff + NB)
                    bsl = slice(bn * NB, (bn + 1) * NB)
                    for jt in range(JT):
                        ps = psum_pool.tile([P, NB], mybir.dt.float32, tag="ps")
                        for it in range(KT):
                            nc.tensor.matmul(
                                ps[:],
                                qts[it][:, jt * P : (jt + 1) * P],
                                sups[it][:, bsl],
                                start=(it == 0),
                                stop=(it == KT - 1),
                            )
                        if jt % 2 == 0:
                            nc.vector.tensor_copy(out=ots[jt][:, osl], in_=ps[:])
                        else:
                            nc.scalar.copy(out=ots[jt][:, osl], in_=ps[:])
                    if goff + NB == gw:
                        g0 = grp_base[gi]
                        for jt in range(JT):
                            nc.sync.dma_start(
                                out=outT[jt * P : (jt + 1) * P, g0 : g0 + gw],
                                in_=ots[jt][:, :gw],
                            )
                col_base += csz
    nc.compile()
    return nc


def _build_t2(
    mode,
    bc=BC,
    chunk=None,
    warmups=None,
):
    """v2 of the transposed-output kernel. Differences from _build_t:

    - Weight reuse: per (chunk, jt, it) ONE stationary Q block feeds G
      consecutive matmuls (one per 512-col b-block, each accumulating into
      its own PSUM bank), instead of swapping stationary every matmul.
    - Warmup matmuls on a memset dummy tile run from t~0 (no DMA
      dependency), so the HAM clock-gate releases before real matmuls and
      the PE isn't idle during the input DMA cold-start.
    - Output: one [128, chunk] fp16 tile per (chunk, jt), stored with a
      single DMA (4 KB/partition contiguous); first/last chunks smaller to
      shorten ramp and final flush.
    """
    dt_in = _DT_IN[mode]
    assert dt_in == mybir.dt.float16
    if chunk is None:
        chunk = int(os.environ.get("KCHUNK", "2048"))
    if warmups is None:
        warmups = int(os.environ.get("KWARM", "34"))
    chl = int(os.environ.get("KCHL", "1024"))
    WNB = int(os.environ.get("KWNB", "256"))  # warmup moving width
    ibufs = int(os.environ.get("KIBUFS", "6"))  # input prefetch depth (chunks)
    obufs = int(os.environ.get("KOBUFS", "3"))  # output tile buffers per j-tile

    nc = bacc.Bacc()
    inpT = nc.dram_tensor("inpT", [D, bc], dt_in, kind="ExternalInput")
    qT = nc.dram_tensor("qT", [D, J], dt_in, kind="ExternalInput")
    outT = nc.dram_tensor("outT", [J, bc], mybir.dt.float16, kind="ExternalOutput")

    NB = 512  # moving free dim per matmul (one PSUM bank)
    JT = J // P  # 4 j-tiles

    # chunk plan: measured-optimal head is a single 1024 chunk (the stream
    # start is gated by ~1.5MB of DMA however it is sliced; finer slicing
    # only buys LDWEIGHTS exposure), wide middle, 1024 last (fast flush)
    head = [int(x) for x in os.environ.get("KHEAD", "1024").split(",") if x]
    plan = []
    rem = bc
    for c in head:
        if rem - c > chl:
            plan.append(c)
            rem -= c
    while rem > (chl if chl else 0):
        c = min(chunk, rem - chl)
        plan.append(c)
        rem -= c
    if rem:
        plan.append(rem)
    assert sum(plan) == bc and all(c % NB == 0 for c in plan), plan

    if warmups:
        # Pre-TileContext warmup matmuls, emitted straight into the `main`
        # block so the PE starts them right after the framework preamble
        # (~5.8us), well before the Tile body's first DMA lands. They warm
        # the HAM clock gate to K=8/8 (2.4 GHz) so real matmuls start
        # un-throttled. No cross-engine sync on purpose: the operand values
        # are irrelevant (the PSUM bank is never read) so reading SBUF
        # before/while gpsimd memsets it is harmless.
        wsb = nc.alloc_sbuf_tensor("wsb", [P, WNB], dt_in)
        nc.gpsimd.memset(wsb[:, :], 0)
        wps = nc.alloc_psum_tensor("wps", [P, WNB], mybir.dt.float32)
        for wi in range(warmups):
            nc.tensor.matmul(
                wps[:, :],
                wsb[:, :P],
                wsb[:, :],
                start=(wi == 0),
                stop=(wi == warmups - 1),
            )

    with tile.TileContext(nc) as tc:
        with (
            tc.tile_pool(name="qpool", bufs=1) as qpool,
            tc.tile_pool(name="inpool", bufs=ibufs) as inpool,
            tc.tile_pool(name="outpool", bufs=obufs) as outpool,
            tc.tile_pool(name="psum", bufs=7, space="PSUM") as psum_pool,
        ):
            # Q.T rows for i-tile `it`: [128i, 512j]; stationary blocks are
            # 128-col slices. Ride the sync HW queue (idle early) in parallel
            # with the input ring. NOT gpsimd (software-DGE, slow).
            qts = []
            for it in range(KT):
                qt_t = qpool.tile([P, J], dt_in, tag=f"qt{it}", name=f"qt{it}")
                nc.sync.dma_start(out=qt_t[:], in_=qT[it * P : (it + 1) * P, :])
                qts.append(qt_t)

            # All input on the scalar ring: per-ring FIFO keeps arrival
            # order == consumption order, and the graduated head chunks keep
            # the first slices from queueing behind bulk prefetch.
            col_base = 0
            for ci, csz in enumerate(plan):
                G = csz // NB
                csl = slice(col_base, col_base + csz)
                sups = []
                for it in range(KT):
                    sup = inpool.tile([P, csz], dt_in, tag=f"sup{it}", name=f"sup{it}")
                    nc.scalar.dma_start(
                        out=sup[:], in_=inpT[it * P : (it + 1) * P, csl]
                    )
                    sups.append(sup)
                for jt in range(JT):
                    pss = [
                        psum_pool.tile([P, NB], mybir.dt.float32, tag="ps", name="ps")
                        for _ in range(G)
                    ]
                    ot = outpool.tile(
                        [P, csz], mybir.dt.float16, tag=f"ot{jt}", name=f"ot{jt}"
                    )
                    for it in range(KT):
                        # one stationary load serves G consecutive matmuls
                        for g in range(G):
                            nc.tensor.matmul(
                                pss[g][:],
                                qts[it][:, jt * P : (jt + 1) * P],
                                sups[it][:, g * NB : (g + 1) * NB],
                                start=(it == 0),
                                stop=(it == KT - 1),
                            )
                    for g in range(G):
                        # split PSUM->SBUF cast-copies across DVE and ACT
                        if g % 2 == 0:
                            nc.vector.tensor_copy(
                                out=ot[:, g * NB : (g + 1) * NB], in_=pss[g][:]
                            )
                        else:
                            nc.scalar.copy(
                                out=ot[:, g * NB : (g + 1) * NB], in_=pss[g][:]
                            )
                    # outputs ride the sync HW ring; the LAST chunk's odd-jt
                    # flushes ride scalar (input is long done), halving the
                    # end-of-stream flush latency
                    last = ci == len(plan) - 1
                    oeng = nc.scalar if (last and jt % 2 == 1) else nc.sync
                    oeng.dma_start(
                        out=outT[jt * P : (jt + 1) * P, csl], in_=ot[:]
                    )
                col_base += csz
    nc.compile()
    return nc


def _build_t3(mode, bc=BC):
    """v3 of the transposed-output kernel: host-interleaved input blocks.

    Host stores the input as blocks of 512 b-columns with all 4 i-slices
    contiguous per partition: inp_blk[p, b, it, c] = inpT[it*128+p, b*512+c],
    so EVERY input DMA moves 4 KB/partition contiguous runs (the early DMA
    rate is packet-size-bound: 1 KB packets reach only ~130 GB/s, 4 KB
    ~210 GB/s). One transfer per block; scalar-ring FIFO; tile-pool bufs
    bound the prefetch depth. Q is likewise a single interleaved transfer.

    Matmul schedule: pairs of blocks (G=2 is enough to hide LDWEIGHTS) with
    NB=512; the first/last blocks run alone with NB=256 (still G=2) so the
    stream can start on half a megabyte and the final flush is small.
    """
    dt_in = _DT_IN[mode]
    assert dt_in == mybir.dt.float16
    warmups = int(os.environ.get("KWARM", "16"))
    WNB = int(os.environ.get("KWNB", "256"))
    n_sing_head = int(os.environ.get("KSINGH", "2"))
    n_sing_tail = int(os.environ.get("KSINGT", "2"))
    BUFS = int(os.environ.get("KBUFS", "8"))

    NBLK = bc // 512  # 32 blocks of 512 b-columns
    assert (NBLK - n_sing_head - n_sing_tail) % 2 == 0

    nc = bacc.Bacc()
    # interleaved layouts (built on host)
    inp_blk = nc.dram_tensor("inp_blk", [P, NBLK * KT * 512], dt_in, kind="ExternalInput")
    q_blk = nc.dram_tensor("q_blk", [P, KT * J], dt_in, kind="ExternalInput")
    outT = nc.dram_tensor("outT", [J, bc], mybir.dt.float16, kind="ExternalOutput")

    JT = J // P

    if warmups:
        wsb = nc.alloc_sbuf_tensor("wsb", [P, WNB], dt_in)
        nc.gpsimd.memset(wsb[:, :], 0)
        wps = nc.alloc_psum_tensor("wps", [P, WNB], mybir.dt.float32)
        for wi in range(warmups):
            nc.tensor.matmul(
                wps[:, :],
                wsb[:, :P],
                wsb[:, :],
                start=(wi == 0),
                stop=(wi == warmups - 1),
            )

    # chunk plan over blocks: singles at head/tail, pairs in the middle
    chunks = [[b] for b in range(n_sing_head)]
    b = n_sing_head
    while b < NBLK - n_sing_tail:
        chunks.append([b, b + 1])
        b += 2
    chunks.extend([[b] for b in range(NBLK - n_sing_tail, NBLK)])

    with tile.TileContext(nc) as tc:
        with (
            tc.tile_pool(name="qpool", bufs=1) as qpool,
            tc.tile_pool(name="inpool", bufs=BUFS) as inpool,
            tc.tile_pool(name="outpool", bufs=2) as outpool,
            tc.tile_pool(name="psum", bufs=7, space="PSUM") as psum_pool,
        ):
            # Q: ONE interleaved transfer on the sync HW queue: 4KB/partition
            # contiguous run -> 4KB packets (per-i-tile slices would be 1KB
            # packets at ~50 GB/s). Never gpsimd: software-DGE, slow.
            qt = qpool.tile([P, KT * J], dt_in, tag="qt", name="qt")
            nc.sync.dma_start(out=qt[:], in_=q_blk[:, :])

            # all input transfers up-front in block order; pool bufs bound
            # the in-flight prefetch window
            blks = []
            for bi in range(NBLK):
                t = inpool.tile([P, KT * 512], dt_in, tag="blk", name="blk")
                nc.scalar.dma_start(
                    out=t[:], in_=inp_blk[:, bi * KT * 512 : (bi + 1) * KT * 512]
                )
                blks.append(t)

            # output groups: wide (2048 cols -> 4KB packets) in the middle,
            # tapered at the end so the final flush after the last matmul is
            # small. Group boundaries align with chunk boundaries.
            ow_plan = [4, 4, 4, 4, 4, 4, 4, 2, 1, 1]  # in blocks; sums to 32
            assert sum(ow_plan) == NBLK
            blk2grp = {}
            gb = 0
            for gi, w in enumerate(ow_plan):
                for off in range(w):
                    blk2grp[gb + off] = (gi, off, w)
                gb += w
            grp_base = {}
            gb = 0
            for gi, w in enumerate(ow_plan):
                grp_base[gi] = gb
                gb += w
            ots = [None] * JT

            copy_i = 0
            for chunk in chunks:
                csz = 512 * len(chunk)
                nb = csz // 2 if len(chunk) == 1 else 512  # NB=256 singles, 512 pairs
                for jt in range(JT):
                    pss = [
                        psum_pool.tile([P, nb], mybir.dt.float32, tag="ps", name="ps")
                        for _ in range(2)
                    ]
                    gi, goff, gw = blk2grp[chunk[0]]
                    if goff == 0 and jt == 0:
                        for j2 in range(JT):
                            ots[j2] = outpool.tile(
                                [P, gw * 512],
                                mybir.dt.float16,
                                tag=f"ot{j2}",
                                name=f"ot{j2}",
                            )
                    for it in range(KT):
                        for g in range(2):
                            if len(chunk) == 1:
                                mv = blks[chunk[0]][:, it * 512 + g * nb : it * 512 + (g + 1) * nb]
                            else:
                                mv = blks[chunk[g]][:, it * 512 : (it + 1) * 512]
                            nc.tensor.matmul(
                                pss[g][:],
                                qt[:, it * J + jt * P : it * J + (jt + 1) * P],
                                mv,
                                start=(it == 0),
                                stop=(it == KT - 1),
                            )
                    for g in range(2):
                        osl = slice(goff * 512 + g * nb, goff * 512 + (g + 1) * nb)
                        # alternate engines by a global counter so DVE and
                        # ACT each carry half the PSUM-drain load
                        if copy_i % 2 == 0:
                            nc.vector.tensor_copy(out=ots[jt][:, osl], in_=pss[g][:])
                        else:
                            nc.scalar.copy(out=ots[jt][:, osl], in_=pss[g][:])
                        copy_i += 1
                    if goff + len(chunk) == gw:
                        g0 = grp_base[gi] * 512
                        nc.sync.dma_start(
                            out=outT[jt * P : (jt + 1) * P, g0 : g0 + gw * 512],
                            in_=ots[jt][:, : gw * 512],
                        )
    nc.compile()
    return nc


def _get_nc(mode):
    if mode not in _compiled:
        if mode == "f16t3":
            _compiled[mode] = _build_t3(mode)
        elif mode == "f16t2":
            _compiled[mode] = _build_t2(mode)
        elif mode == "f16t":
            _compiled[mode] = _build_t(mode)
        else:
            _compiled[mode] = _build(mode)
    return _compiled[mode]


def kernel(inp: np.ndarray, weight: np.ndarray) -> np.ndarray:
    global LAST_RESULTS
    mode = MODE
    nc = _get_nc(mode)

    w = np.asarray(weight, dtype=np.float32) + np.float32(1e-8)
    Q = np.linalg.qr(w)[0].astype(np.float32)  # [J, D] == [512, 512]
    np_dt = _np_in_dtype(mode)

    inp = np.asarray(inp, dtype=np.float32)
    inpT = inp.T  # [D, B] view

    QT = Q.T  # QT[i, j] = Q[j, i]
    in_maps = []
    if mode.endswith("x3"):
        qt_hi = QT.astype(np_dt)
        qt_lo = (QT - qt_hi.astype(np.float32)).astype(np_dt)
        for c in range(N_CORES):
            sl = inpT[:, c * BC : (c + 1) * BC].astype(np.float32)
            hi = sl.astype(np_dt)
            lo = (sl - hi.astype(np.float32)).astype(np_dt)
            in_maps.append(
                {"inpT_hi": hi, "inpT_lo": lo, "qT_hi": qt_hi, "qT_lo": qt_lo}
            )
    elif mode == "f16t3":
        # interleaved layouts: q_blk[p, it*512+j] = QT[it*128+p, j];
        # inp_blk[p, (b*4+it)*512+c] = inpT[it*128+p, b*512+c]
        q_blk = np.ascontiguousarray(
            QT.astype(np_dt).reshape(KT, P, J).transpose(1, 0, 2)
        ).reshape(P, KT * J)
        for c in range(N_CORES):
            sl = inpT[:, c * BC : (c + 1) * BC].astype(np_dt)  # [D, BC]
            blk = np.ascontiguousarray(
                sl.reshape(KT, P, BC // 512, 512).transpose(1, 2, 0, 3)
            ).reshape(P, BC * KT)
            in_maps.append({"inp_blk": blk, "q_blk": q_blk})
    else:
        qt16 = np.ascontiguousarray(QT).astype(np_dt)
        for c in range(N_CORES):
            in_maps.append(
                {"inpT": inpT[:, c * BC : (c + 1) * BC].astype(np_dt), "qT": qt16}
            )

    # First execution of a freshly compiled NEFF occasionally dies with
    # NRT_EXEC_UNIT_UNRECOVERABLE (transient, esp. with profiling on);
    # a straight retry has always succeeded.
    last_exc = None
    for _attempt in range(3):
        try:
            res = run_bass_kernel_spmd(nc, in_maps, list(range(N_CORES)))
            break
        except Exception as e:  # noqa: BLE001
            last_exc = e
            import time as _time

            _time.sleep(2.0)
    else:
        raise last_exc
    LAST_RESULTS = res
    if mode in ("f16t", "f16t2", "f16t3"):
        out = np.empty((B, J), dtype=np.float32)
        for c in range(N_CORES):
            # outT [J, BC] fp16 -> out rows [c*BC:(c+1)*BC] fp32
            out[c * BC : (c + 1) * BC, :] = res.results[c]["outT"].T
        return out
    return np.concatenate([res.results[c]["out"] for c in range(N_CORES)], axis=0)

